# revision 43
# baseline (speedup 1.0000x reference)
"""v2 variant: pair-major phases, 5-way x2n chunks, SWDGE output DMAs.

See kernel.py for the full design commentary; this file differs only in:
  - x2te layout [R_hi | R_lo | s] with cc-major chunks (pair-0 starter,
    cc2-7, cc8-15), so phase S runs pair-major (lhsT amortized over the
    4 banks) as one continuous PE burst after the stream lands;
  - x2n in 5 chunks sized 2/4/4/4/2 sc for finer phase-T pacing;
  - outputs tt [32, 2048] + tl [16, 4] go out via the GpSimd SWDGE
    queue (each waits on a single engine: DVE for tt, ScalarE for tl).
"""

import sys

for _p in ("/root/.axon_site/_ro/trn_rl_repo", "/opt/trn_rl_repo"):
    if _p not in sys.path:
        sys.path.append(_p)

import numpy as np
import ml_dtypes

import concourse.bass as bass
import concourse.tile as tile
from concourse import mybir
from concourse.bass_utils import run_bass_kernel_spmd
from concourse.tile_rust import add_dep_helper

NCORES = 8
S_FULL = 16384
C = 2048
H = 16
H2 = 2 * H
J = 128
HJ = H * J
ODIM = 512
S_LOC = S_FULL // NCORES

E4 = mybir.dt.float8e4
BF = mybir.dt.bfloat16
F32 = mybir.dt.float32
F32R = mybir.dt.float32r
INV_SQRT_K = 1.0 / float(np.sqrt(128.0))
SCALE_R = 1024.0
EXP_BIAS = -2.0

NB = 512
CH = C // 128
RW = 2 * H
CE = C + RW

_FP8_NP = ml_dtypes.float8_e4m3


def _build_program() -> bass.Bass:
    nc = bass.Bass()
    _bias_sb = nc.alloc_sbuf_tensor("const-f32-bias", [J, 1], F32)
    nc.gpsimd.memset(_bias_sb.ap(), EXP_BIAS)
    nc.const_aps.aps[(F32, EXP_BIAS)] = _bias_sb.ap()

    t_in = {
        "x2te": nc.dram_tensor("x2te", [J, CH, CE], E4, kind="ExternalInput"),
        "x2n": nc.dram_tensor("x2n", [J, CH, C], E4, kind="ExternalInput"),
    }
    t_out = {
        "tt": nc.dram_tensor("tt", [H2, C], F32, kind="ExternalOutput"),
        "tl": nc.dram_tensor("tl", [H, 4], F32, kind="ExternalOutput"),
    }

    x2te_v = t_in["x2te"][:, :, :]
    x2n_v = t_in["x2n"][:, :, :]

    DR = mybir.MatmulPerfMode.DoubleRow

    with tile.TileContext(nc) as tc:
        with (
            tc.tile_pool(name="singles", bufs=1) as singles,
            tc.tile_pool(name="sa", bufs=1) as sa,
            tc.tile_pool(name="sb", bufs=1) as sbp,
            tc.tile_pool(name="psbig", bufs=4, space="PSUM") as psbig,
            tc.tile_pool(name="pstr", bufs=4, space="PSUM") as pstr,
        ):
            ep_targets = []

            xtr_a = sa.tile([J, 2, CE], E4, tag="xtr_a")
            ep_targets.append(nc.sync.dma_start(out=xtr_a, in_=x2te_v[:, 0:2, :]))
            xtr_b = sa.tile([J, 6, CE], E4, tag="xtr_b")
            ep_targets.append(nc.sync.dma_start(out=xtr_b, in_=x2te_v[:, 2:8, :]))
            xtr_c = sa.tile([J, 8, CE], E4, tag="xtr_c")
            ep_targets.append(nc.scalar.dma_start(out=xtr_c, in_=x2te_v[:, 8:16, :]))
            xn_a = sbp.tile([J, 2, C], E4, tag="xn_a")
            ep_targets.append(nc.sync.dma_start(out=xn_a, in_=x2n_v[:, 0:2, :]))
            xn_b = sbp.tile([J, 4, C], E4, tag="xn_b")
            ep_targets.append(nc.scalar.dma_start(out=xn_b, in_=x2n_v[:, 2:6, :]))
            # third concurrent stream: the GpSimd SWDGE queue (whose drain
            # cost this variant already pays for the output DMAs) carries
            # the middle x2n chunk, relieving both HW-DGE rings
            xn_c = sbp.tile([J, 4, C], E4, tag="xn_c")
            ep_targets.append(nc.gpsimd.dma_start(out=xn_c, in_=x2n_v[:, 6:10, :]))
            xn_d = sbp.tile([J, 4, C], E4, tag="xn_d")
            ep_targets.append(nc.scalar.dma_start(out=xn_d, in_=x2n_v[:, 10:14, :]))
            xn_e = sbp.tile([J, 2, C], E4, tag="xn_e")
            ep_targets.append(nc.sync.dma_start(out=xn_e, in_=x2n_v[:, 14:16, :]))

            ident16 = singles.tile([H, H], BF)
            nc.gpsimd.memset(ident16, 0.0)
            nc.gpsimd.affine_select(
                out=ident16,
                in_=ident16,
                compare_op=mybir.AluOpType.not_equal,
                fill=1.0,
                base=0,
                pattern=[[-1, H]],
                channel_multiplier=1,
            )
            ident32 = singles.tile([H2, H], F32)
            nc.gpsimd.memset(ident32, 0.0)
            nc.gpsimd.affine_select(
                out=ident32,
                in_=ident32,
                compare_op=mybir.AluOpType.not_equal,
                fill=1.0,
                base=0,
                pattern=[[-1, H]],
                channel_multiplier=1,
            )
            i_pool = nc.gpsimd.affine_select(
                out=ident32,
                in_=ident32,
                compare_op=mybir.AluOpType.not_equal,
                fill=1.0,
                base=-H,
                pattern=[[-1, H]],
                channel_multiplier=1,
            )
            ident32v = singles.tile([H2, H], F32R)
            nc.vector.tensor_copy(out=ident32v, in_=ident32)

            ps = [
                psbig.tile([H2, NB], F32, tag="big", name=f"ps_s{m}")
                for m in range(4)
            ]

            def xtr_tile(j):
                if j == 0:
                    return xtr_a, 0
                if j <= 3:
                    return xtr_b, 2 * (j - 1)
                return xtr_c, 2 * (j - 4)

            for j in range(8):
                xt, o = xtr_tile(j)
                for m in range(4):
                    nc.tensor.matmul(
                        ps[m][:H2, :],
                        lhsT=xt[:, o : o + 2, 0:RW],
                        rhs=xt[:, o : o + 2, RW + m * NB : RW + (m + 1) * NB],
                        start=(j == 0),
                        stop=(j == 7),
                        perf_mode=DR,
                    )

            ssb = singles.tile([H2, S_LOC], F32R)
            for m in range(4):
                nc.vector.tensor_copy(
                    out=ssb[:, m * NB : (m + 1) * NB], in_=ps[m][:H2, :]
                )
            mg = [
                psbig.tile([H2, NB], F32, tag="big", name=f"ps_mg{m}")
                for m in range(4)
            ]
            for m in range(4):
                nc.tensor.matmul(
                    mg[m][:H, :],
                    lhsT=ident32v[:, :],
                    rhs=ssb[:, m * NB : (m + 1) * NB],
                    start=True,
                    stop=True,
                )
            Psb = singles.tile([H, S_LOC], BF)
            tl_sb = singles.tile([H, 4], F32)
            i_exp = None
            for m in range(4):
                i_exp = nc.scalar.activation(
                    out=Psb[:, m * NB : (m + 1) * NB],
                    in_=mg[m][:H, :],
                    func=mybir.ActivationFunctionType.Exp,
                    scale=1.0 / SCALE_R,
                    bias=EXP_BIAS,
                    accum_out=tl_sb[:, m : m + 1],
                )

            PT = singles.tile([J, CH, H2], E4)
            i_dve = None
            for sb in range(CH):
                blk = slice(sb * J, (sb + 1) * J)
                pst = pstr.tile([J, H], BF, tag="tr", name=f"tr{sb}")
                nc.tensor.transpose(pst, Psb[:, blk], ident16)
                nc.vector.tensor_copy(out=PT[:, sb, 0:H], in_=pst)
                i_dve = nc.vector.tensor_sub(
                    out=PT[:, sb, H:H2], in0=pst, in1=PT[:, sb, 0:H]
                )

            pt = [
                psbig.tile([H2, NB], F32, tag="big", name=f"ps_t{m}")
                for m in range(4)
            ]

            def xn_tile(j):
                if j == 0:
                    return xn_a, 0
                if j <= 2:
                    return xn_b, 2 * (j - 1)
                if j <= 4:
                    return xn_c, 2 * (j - 3)
                if j <= 6:
                    return xn_d, 2 * (j - 5)
                return xn_e, 0

            i_pe = None
            for j in range(8):
                xn, o = xn_tile(j)
                for m in range(4):
                    i_pe = nc.tensor.matmul(
                        pt[m][:H2, :],
                        lhsT=PT[:, 2 * j : 2 * j + 2, :],
                        rhs=xn[:, o : o + 2, m * NB : (m + 1) * NB],
                        start=(j == 0),
                        stop=(j == 7),
                        perf_mode=DR,
                    )

            tt_sb = singles.tile([H2, C], F32)
            i_tcopies = []
            for m in range(4):
                i_tcopies.append(
                    nc.vector.tensor_copy(
                        out=tt_sb[:, m * NB : (m + 1) * NB], in_=pt[m][:H2, :]
                    )
                )
            i_out_l = nc.gpsimd.dma_start(out=t_out["tl"][:, :], in_=tl_sb)
            i_out_t = nc.gpsimd.dma_start(out=t_out["tt"][:, :], in_=tt_sb)
            i_gp_last = nc.gpsimd.nop(nofuse=True, hint="dep")

            ep_targets += [
                i_pool, i_exp, i_dve, i_pe, *i_tcopies,
                i_out_l, i_out_t, i_gp_last,
            ]
            for t in ep_targets:
                n = nc.sync.nop(nofuse=True, hint="dep")
                add_dep_helper(n.ins, t.ins, reason="drain-funnel")

    return nc


_NC_CACHE = None


def _get_nc() -> bass.Bass:
    global _NC_CACHE
    if _NC_CACHE is None:
        _NC_CACHE = _build_program()
    return _NC_CACHE


def _prep_in_maps(x1, x2, Wq, Wk):
    x1 = np.asarray(x1, np.float32)
    x2 = np.asarray(x2, np.float32)
    Wq = np.asarray(Wq, np.float32)
    Wk = np.asarray(Wk, np.float32)

    q = (Wq @ x1) * INV_SQRT_K
    R = np.einsum("hj,hjc->ch", q.reshape(H, J), Wk.reshape(H, J, C))
    Rs = np.clip(R * SCALE_R, -240.0, 240.0).astype(np.float32)
    Rhi = Rs.astype(_FP8_NP)
    Rlo = (Rs - Rhi.astype(np.float32)).astype(_FP8_NP)
    Rhi_p = np.ascontiguousarray(Rhi.reshape(CH, J, H).transpose(1, 0, 2))
    Rlo_p = np.ascontiguousarray(Rlo.reshape(CH, J, H).transpose(1, 0, 2))

    in_maps = []
    for c in range(NCORES):
        shard = x2[c * S_LOC : (c + 1) * S_LOC]
        x2te = np.empty((J, CH, CE), dtype=_FP8_NP)
        x2te[:, :, 0:H] = Rhi_p
        x2te[:, :, H:RW] = Rlo_p
        x2te[:, :, RW:] = (
            shard.T.reshape(CH, J, S_LOC).transpose(1, 0, 2).astype(_FP8_NP)
        )
        x2n_c = np.ascontiguousarray(
            shard.reshape(CH, J, C).transpose(1, 0, 2)
        ).astype(_FP8_NP)
        in_maps.append({"x2te": x2te, "x2n": x2n_c})
    return in_maps


def _merge(results, Wv, Wo, bo):
    Wv = np.asarray(Wv, np.float32)
    Wo = np.asarray(Wo, np.float32)
    bo = np.asarray(bo, np.float32)
    t_tot = np.zeros((H, C), np.float64)
    l_tot = np.zeros(H, np.float64)
    for r in results:
        tt = r["tt"].astype(np.float64)
        t_tot += tt[:H] + tt[H:]
        l_tot += r["tl"].astype(np.float64).sum(axis=1)
    tn = t_tot / l_tot[:, None]
    u = np.einsum("hc,hjc->hj", tn, Wv.astype(np.float64).reshape(H, J, C))
    out = u.reshape(HJ) @ Wo.T.astype(np.float64) + bo.astype(np.float64)
    return out.astype(np.float32).reshape(1, ODIM)


def kernel(x1, x2, Wq, Wk, Wv, Wo, bo):
    nc = _get_nc()
    in_maps = _prep_in_maps(x1, x2, Wq, Wk)
    res = run_bass_kernel_spmd(nc, in_maps, list(range(NCORES)))
    return _merge(res.results, Wv, Wo, bo)


def run_traced(x1, x2, Wq, Wk, Wv, Wo, bo, **trace_kwargs):
    nc = _get_nc()
    in_maps = _prep_in_maps(x1, x2, Wq, Wk)
    res = run_bass_kernel_spmd(
        nc, in_maps, list(range(NCORES)), trace=True, **trace_kwargs
    )
    return _merge(res.results, Wv, Wo, bo), res


# revision 49
# speedup vs baseline: 1.1241x; 1.1241x over previous
"""Trainium2 Bass kernel for nn_CrossAttention_14207751815513.

Single-query cross-attention:
    q = x1 @ Wq.T                 (one query per head)
    k = x2 @ Wk.T ; v = x2 @ Wv.T
    attn_h = softmax(q_h . k_h / sqrt(128))
    out = concat_h(attn_h @ v_h) @ Wo.T + bo

Because there is exactly ONE query, the K and V projections collapse
algebraically (associativity):
    scores_h = x2 @ r_h,  r_h = Wk_h.T q_h / sqrt(128)   -- no k materialization
    out_h    = Wv_h @ (x2.T p_h) / l_h                   -- no v materialization
with p = exp(scores + EXP_BIAS) (the constant bias cancels in t/l) and
l_h = sum_s p_h[s].

Sharding: the sequence dim (16384) is split across the 8 NeuronCores
(2048 rows each).  All O(1)-in-S work (q, R, Wv matvec, Wo + bias) is
host-side glue; the O(S*C) work runs on device.

fp8 design (half the HBM bytes of the bf16 version, double the PE rate;
~1.9x faster end to end):
  - x2 is shipped in BOTH orientations as fp8e4 (e4m3): x2te [p, cc,
    R|s] (transposed, c-on-partitions, R embedded at the head of each
    row) and x2n [p, sc, c].  ~8.4MB/core; this stream is the roofline.
  - All big matmuls use fp8e4 DoubleRow perf mode: 2 k-tiles (256-deep
    contraction) per instruction, 0.5 cycles/row.
  - e4m3's 3 mantissa bits are too coarse for R (the folded query) and
    P (the exp'd scores); both use an UNSCALED two-term hi+lo split:
    v ~ e4(v) + e4(v - e4(v)).  Measured end-to-end rel err vs the f32
    reference: ~1.27e-2 (gate: 2e-2).
  - The hi/lo terms are packed side by side in the STATIONARY free dim
    (lhsT [128, 2, 32]), so one DoubleRow matmul emits [32, 512]: rows
    0:16 are the hi partial, rows 16:32 the lo partial.  Matmul cost
    scales only with streamed columns, so the split is FREE on the PE
    (32 matmuls per phase instead of 128).  Phase S rows are merged
    (hi+lo) before exp by a tiny f32r matmul with a stacked identity
    [I16; I16] (DVE ops cannot read two PSUM operands, and engine
    partition offsets must be 32-aligned, so a PE reduction it is);
    phase T rows are merged on the host.
  - Phases are PAIR-major (j outer, bank inner): the lhsT stays fixed
    across the 4 bank matmuls (LDWEIGHTS amortized) and PSUM banks
    alternate.  NOTE: the PE clock ramps (~1.2GHz cold, ~2.4GHz after
    ~2-3us of continuous work), so long uninterrupted matmul bursts
    are measurably faster than finely stall-interleaved ones.
  - exp is scalar.activation(Exp, scale=1/1024, bias=-2) straight from
    the merged PSUM (scores carry a 1024x scale for R's fp8 range); its
    accum_out emits the per-head softmax denominator l for free.
  - P: exp writes bf16; after the PE transpose, PT[:, sb, 0:16] =
    e4(PT) (DVE copy) and PT[:, sb, 16:32] = PT - e4(PT) (DVE sub).

Per-core device program:
  S  : scores32[hi|lo, s] = sum_c [Rhi|Rlo][c, :] x2t[c, s]   (8 cc-pairs
       x 4 banks, DoubleRow)
  mrg: scores[h, s] = scores32[h, s] + scores32[16+h, s]      (DVE copy to
       SBUF f32r + identity matmul)
  exp: P = exp(scores/1024 - 2), l = rowsum(P)                (ScalarE)
  tr : P [16, 2048] -> PT [128, 16 sb, 32]; hi/lo split       (PE + DVE)
  T  : t32[hi|lo, c] = sum_s [PTh|PTl][s, :] x2n[s, c]        (8 sc-pairs
       x 4 banks, DoubleRow)
Outputs per core: tt [32, 2048] f32 (hi/lo t partials), tl [16, 4] f32
(l partials).  Host: t = tt[:16] + tt[16:], l = tl.sum, normalize by l,
apply Wv per head, then Wo + bo.

Sync-wait note: this backend disables DynamicDMA, so every HW-DGE DMA
lowers to a pseudo-direct DMA that supports at most ONE semaphore wait
("Too many sync wait commands" in walrus codegen otherwise); the
Activation queue, DVE TensorCopy, and the matmul LdWeights slot also
support only ONE wait each (same-engine RAW/WAW deps are emitted as
real semaphore waits and count against the budget).  The program is
structured so no instruction exceeds its budget:
  - every streamed tile is a fresh buffer (unique pool tag, no reuse)
    so stream DMAs carry no WAR/WAW waits;
  - exactly 8 HW-DGE DMAs are issued (the 8 HW-DGE semaphore slots are
    assigned globally round-robin across both rings): 3 x2te chunks
    (pair-0 starter + cc2-7 on SP, cc8-15 on Act) and 5 x2n chunks
    (2/4/4/4/2 sc, alternating rings) — ~4.2MB per ring, x2te leading
    both FIFOs since phase S gates exp -> PT -> phase T; chunk rows are
    kept CONTIGUOUS per partition (multi-KB descriptor runs) — padding
    or striding the SBUF tiles fragments the DMA and slows the stream;
  - the outputs go out via the GpSimd SWDGE queue, each waiting on a
    single engine (DVE for tt, ScalarE for tl);
  - the merge matmul's lhsT identity is bounced through a DVE copy so
    its LdWeights wait collapses to the same DVE sem as its rhs;
  - EXP_BIAS is registered as a const AP (mirroring the builtin consts)
    so the exp activation carries only its PE RAW wait;
  - the end-of-context Drain gets a sem wait for every proc the SP
    engine hasn't directly observed (the wait clock is not
    transitive), so an epilogue of single-dep SP nops makes SP observe
    each DMA and each engine's last instruction first (incl. a
    trailing GpSimd nop that covers the SWDGE DMA instructions).
"""

import sys

for _p in ("/root/.axon_site/_ro/trn_rl_repo", "/opt/trn_rl_repo"):
    if _p not in sys.path:
        sys.path.append(_p)

import numpy as np
import ml_dtypes

import concourse.bass as bass
import concourse.tile as tile
from concourse import mybir
from concourse.bass_utils import run_bass_kernel_spmd
from concourse.tile_rust import add_dep_helper

NCORES = 8
S_FULL = 16384
C = 2048
H = 16
H2 = 2 * H
J = 128
HJ = H * J
ODIM = 512
S_LOC = S_FULL // NCORES

E4 = mybir.dt.float8e4
BF = mybir.dt.bfloat16
F32 = mybir.dt.float32
F32R = mybir.dt.float32r
INV_SQRT_K = 1.0 / float(np.sqrt(128.0))
SCALE_R = 1024.0
EXP_BIAS = -2.0

NB = 512
CH = C // 128
RW = 2 * H
CE = C + RW

_FP8_NP = ml_dtypes.float8_e4m3


def _build_program() -> bass.Bass:
    nc = bass.Bass()
    _bias_sb = nc.alloc_sbuf_tensor("const-f32-bias", [J, 1], F32)
    nc.gpsimd.memset(_bias_sb.ap(), EXP_BIAS)
    nc.const_aps.aps[(F32, EXP_BIAS)] = _bias_sb.ap()

    t_in = {
        "x2te": nc.dram_tensor("x2te", [J, CH, CE], E4, kind="ExternalInput"),
        "x2n": nc.dram_tensor("x2n", [J, CH, C], E4, kind="ExternalInput"),
    }
    t_out = {
        "tt": nc.dram_tensor("tt", [H2, C], F32, kind="ExternalOutput"),
        "tl": nc.dram_tensor("tl", [H, 4], F32, kind="ExternalOutput"),
    }

    x2te_v = t_in["x2te"][:, :, :]
    x2n_v = t_in["x2n"][:, :, :]

    DR = mybir.MatmulPerfMode.DoubleRow

    with tile.TileContext(nc) as tc:
        with (
            tc.tile_pool(name="singles", bufs=1) as singles,
            tc.tile_pool(name="sa", bufs=1) as sa,
            tc.tile_pool(name="sb", bufs=1) as sbp,
            tc.tile_pool(name="psbig", bufs=4, space="PSUM") as psbig,
            tc.tile_pool(name="pstr", bufs=4, space="PSUM") as pstr,
        ):
            ep_targets = []

            xtr_a = sa.tile([J, 2, CE], E4, tag="xtr_a")
            ep_targets.append(nc.sync.dma_start(out=xtr_a, in_=x2te_v[:, 0:2, :]))
            xtr_b = sa.tile([J, 6, CE], E4, tag="xtr_b")
            ep_targets.append(nc.sync.dma_start(out=xtr_b, in_=x2te_v[:, 2:8, :]))
            xtr_c = sa.tile([J, 8, CE], E4, tag="xtr_c")
            ep_targets.append(nc.scalar.dma_start(out=xtr_c, in_=x2te_v[:, 8:16, :]))
            xn_a = sbp.tile([J, 2, C], E4, tag="xn_a")
            ep_targets.append(nc.sync.dma_start(out=xn_a, in_=x2n_v[:, 0:2, :]))
            xn_b = sbp.tile([J, 4, C], E4, tag="xn_b")
            ep_targets.append(nc.scalar.dma_start(out=xn_b, in_=x2n_v[:, 2:6, :]))
            xn_c = sbp.tile([J, 4, C], E4, tag="xn_c")
            ep_targets.append(nc.sync.dma_start(out=xn_c, in_=x2n_v[:, 6:10, :]))
            xn_d = sbp.tile([J, 4, C], E4, tag="xn_d")
            ep_targets.append(nc.scalar.dma_start(out=xn_d, in_=x2n_v[:, 10:14, :]))
            xn_e = sbp.tile([J, 2, C], E4, tag="xn_e")
            ep_targets.append(nc.sync.dma_start(out=xn_e, in_=x2n_v[:, 14:16, :]))

            ident16 = singles.tile([H, H], BF)
            nc.gpsimd.memset(ident16, 0.0)
            nc.gpsimd.affine_select(
                out=ident16,
                in_=ident16,
                compare_op=mybir.AluOpType.not_equal,
                fill=1.0,
                base=0,
                pattern=[[-1, H]],
                channel_multiplier=1,
            )
            ident32 = singles.tile([H2, H], F32)
            nc.gpsimd.memset(ident32, 0.0)
            nc.gpsimd.affine_select(
                out=ident32,
                in_=ident32,
                compare_op=mybir.AluOpType.not_equal,
                fill=1.0,
                base=0,
                pattern=[[-1, H]],
                channel_multiplier=1,
            )
            i_pool = nc.gpsimd.affine_select(
                out=ident32,
                in_=ident32,
                compare_op=mybir.AluOpType.not_equal,
                fill=1.0,
                base=-H,
                pattern=[[-1, H]],
                channel_multiplier=1,
            )
            ident32v = singles.tile([H2, H], F32R)
            nc.vector.tensor_copy(out=ident32v, in_=ident32)

            ps = [
                psbig.tile([H2, NB], F32, tag="big", name=f"ps_s{m}")
                for m in range(4)
            ]

            def xtr_tile(j):
                if j == 0:
                    return xtr_a, 0
                if j <= 3:
                    return xtr_b, 2 * (j - 1)
                return xtr_c, 2 * (j - 4)

            for j in range(8):
                xt, o = xtr_tile(j)
                for m in range(4):
                    nc.tensor.matmul(
                        ps[m][:H2, :],
                        lhsT=xt[:, o : o + 2, 0:RW],
                        rhs=xt[:, o : o + 2, RW + m * NB : RW + (m + 1) * NB],
                        start=(j == 0),
                        stop=(j == 7),
                        perf_mode=DR,
                    )

            ssb = singles.tile([H2, S_LOC], F32R)
            for m in range(4):
                nc.vector.tensor_copy(
                    out=ssb[:, m * NB : (m + 1) * NB], in_=ps[m][:H2, :]
                )
            mg = [
                psbig.tile([H2, NB], F32, tag="big", name=f"ps_mg{m}")
                for m in range(4)
            ]
            for m in range(4):
                nc.tensor.matmul(
                    mg[m][:H, :],
                    lhsT=ident32v[:, :],
                    rhs=ssb[:, m * NB : (m + 1) * NB],
                    start=True,
                    stop=True,
                )
            Psb = singles.tile([H, S_LOC], BF)
            tl_sb = singles.tile([H, 4], F32)
            i_exp = None
            for m in range(4):
                i_exp = nc.scalar.activation(
                    out=Psb[:, m * NB : (m + 1) * NB],
                    in_=mg[m][:H, :],
                    func=mybir.ActivationFunctionType.Exp,
                    scale=1.0 / SCALE_R,
                    bias=EXP_BIAS,
                    accum_out=tl_sb[:, m : m + 1],
                )

            PT = singles.tile([J, CH, H2], E4)
            i_dve = None
            for sb in range(CH):
                blk = slice(sb * J, (sb + 1) * J)
                pst = pstr.tile([J, H], BF, tag="tr", name=f"tr{sb}")
                nc.tensor.transpose(pst, Psb[:, blk], ident16)
                nc.vector.tensor_copy(out=PT[:, sb, 0:H], in_=pst)
                i_dve = nc.vector.tensor_sub(
                    out=PT[:, sb, H:H2], in0=pst, in1=PT[:, sb, 0:H]
                )

            pt = [
                psbig.tile([H2, NB], F32, tag="big", name=f"ps_t{m}")
                for m in range(4)
            ]

            def xn_tile(j):
                if j == 0:
                    return xn_a, 0
                if j <= 2:
                    return xn_b, 2 * (j - 1)
                if j <= 4:
                    return xn_c, 2 * (j - 3)
                if j <= 6:
                    return xn_d, 2 * (j - 5)
                return xn_e, 0

            i_pe = None
            for j in range(8):
                xn, o = xn_tile(j)
                for m in range(4):
                    i_pe = nc.tensor.matmul(
                        pt[m][:H2, :],
                        lhsT=PT[:, 2 * j : 2 * j + 2, :],
                        rhs=xn[:, o : o + 2, m * NB : (m + 1) * NB],
                        start=(j == 0),
                        stop=(j == 7),
                        perf_mode=DR,
                    )

            tt_sb = singles.tile([H2, C], F32)
            i_tcopies = []
            for m in range(4):
                i_tcopies.append(
                    nc.vector.tensor_copy(
                        out=tt_sb[:, m * NB : (m + 1) * NB], in_=pt[m][:H2, :]
                    )
                )
            i_out_l = nc.gpsimd.dma_start(out=t_out["tl"][:, :], in_=tl_sb)
            i_out_t = nc.gpsimd.dma_start(out=t_out["tt"][:, :], in_=tt_sb)
            i_gp_last = nc.gpsimd.nop(nofuse=True, hint="dep")

            ep_targets += [
                i_pool, i_exp, i_dve, i_pe, *i_tcopies,
                i_out_l, i_out_t, i_gp_last,
            ]
            for t in ep_targets:
                n = nc.sync.nop(nofuse=True, hint="dep")
                add_dep_helper(n.ins, t.ins, reason="drain-funnel")

    return nc


_NC_CACHE = None


def _get_nc() -> bass.Bass:
    global _NC_CACHE
    if _NC_CACHE is None:
        _NC_CACHE = _build_program()
    return _NC_CACHE


def _prep_in_maps(x1, x2, Wq, Wk):
    x1 = np.asarray(x1, np.float32)
    x2 = np.asarray(x2, np.float32)
    Wq = np.asarray(Wq, np.float32)
    Wk = np.asarray(Wk, np.float32)

    q = (Wq @ x1) * INV_SQRT_K
    R = np.einsum("hj,hjc->ch", q.reshape(H, J), Wk.reshape(H, J, C))
    Rs = np.clip(R * SCALE_R, -240.0, 240.0).astype(np.float32)
    Rhi = Rs.astype(_FP8_NP)
    Rlo = (Rs - Rhi.astype(np.float32)).astype(_FP8_NP)
    Rhi_p = np.ascontiguousarray(Rhi.reshape(CH, J, H).transpose(1, 0, 2))
    Rlo_p = np.ascontiguousarray(Rlo.reshape(CH, J, H).transpose(1, 0, 2))

    in_maps = []
    for c in range(NCORES):
        shard = x2[c * S_LOC : (c + 1) * S_LOC]
        x2te = np.empty((J, CH, CE), dtype=_FP8_NP)
        x2te[:, :, 0:H] = Rhi_p
        x2te[:, :, H:RW] = Rlo_p
        x2te[:, :, RW:] = (
            shard.T.reshape(CH, J, S_LOC).transpose(1, 0, 2).astype(_FP8_NP)
        )
        x2n_c = np.ascontiguousarray(
            shard.reshape(CH, J, C).transpose(1, 0, 2)
        ).astype(_FP8_NP)
        in_maps.append({"x2te": x2te, "x2n": x2n_c})
    return in_maps


def _merge(results, Wv, Wo, bo):
    Wv = np.asarray(Wv, np.float32)
    Wo = np.asarray(Wo, np.float32)
    bo = np.asarray(bo, np.float32)
    t_tot = np.zeros((H, C), np.float64)
    l_tot = np.zeros(H, np.float64)
    for r in results:
        tt = r["tt"].astype(np.float64)
        t_tot += tt[:H] + tt[H:]
        l_tot += r["tl"].astype(np.float64).sum(axis=1)
    tn = t_tot / l_tot[:, None]
    u = np.einsum("hc,hjc->hj", tn, Wv.astype(np.float64).reshape(H, J, C))
    out = u.reshape(HJ) @ Wo.T.astype(np.float64) + bo.astype(np.float64)
    return out.astype(np.float32).reshape(1, ODIM)


def kernel(x1, x2, Wq, Wk, Wv, Wo, bo):
    nc = _get_nc()
    in_maps = _prep_in_maps(x1, x2, Wq, Wk)
    res = run_bass_kernel_spmd(nc, in_maps, list(range(NCORES)))
    return _merge(res.results, Wv, Wo, bo)


def run_traced(x1, x2, Wq, Wk, Wv, Wo, bo, **trace_kwargs):
    nc = _get_nc()
    in_maps = _prep_in_maps(x1, x2, Wq, Wk)
    res = run_bass_kernel_spmd(
        nc, in_maps, list(range(NCORES)), trace=True, **trace_kwargs
    )
    return _merge(res.results, Wv, Wo, bo), res


# revision 52
# speedup vs baseline: 1.1884x; 1.0572x over previous
"""v2 variant: pair-major phases, 5-way x2n chunks, SWDGE output DMAs.

See kernel.py for the full design commentary; this file differs only in:
  - x2te layout [R_hi | R_lo | s] with cc-major chunks (pair-0 starter,
    cc2-7, cc8-15), so phase S runs pair-major (lhsT amortized over the
    4 banks) as one continuous PE burst after the stream lands;
  - x2n in 5 chunks sized 2/4/4/4/2 sc for finer phase-T pacing;
  - outputs tt [32, 2048] + tl [16, 4] go out via the GpSimd SWDGE
    queue (each waits on a single engine: DVE for tt, ScalarE for tl).
"""

import sys

for _p in ("/root/.axon_site/_ro/trn_rl_repo", "/opt/trn_rl_repo"):
    if _p not in sys.path:
        sys.path.append(_p)

import numpy as np
import ml_dtypes

import concourse.bass as bass
import concourse.tile as tile
from concourse import mybir
from concourse.bass_utils import run_bass_kernel_spmd
from concourse.tile_rust import add_dep_helper

NCORES = 8
S_FULL = 16384
C = 2048
H = 16
H2 = 2 * H
J = 128
HJ = H * J
ODIM = 512
S_LOC = S_FULL // NCORES

E4 = mybir.dt.float8e4
BF = mybir.dt.bfloat16
F32 = mybir.dt.float32
F32R = mybir.dt.float32r
INV_SQRT_K = 1.0 / float(np.sqrt(128.0))
SCALE_R = 1024.0
EXP_BIAS = -2.0

NB = 512
CH = C // 128
RW = 2 * H
CE = C + RW

_FP8_NP = ml_dtypes.float8_e4m3


def _build_program() -> bass.Bass:
    nc = bass.Bass()
    _bias_sb = nc.alloc_sbuf_tensor("const-f32-bias", [J, 1], F32)
    nc.gpsimd.memset(_bias_sb.ap(), EXP_BIAS)
    nc.const_aps.aps[(F32, EXP_BIAS)] = _bias_sb.ap()

    t_in = {
        "x2te": nc.dram_tensor("x2te", [J, CH, CE], E4, kind="ExternalInput"),
        "x2n": nc.dram_tensor("x2n", [J, CH, C], E4, kind="ExternalInput"),
    }
    t_out = {
        "tt": nc.dram_tensor("tt", [H2, C], F32, kind="ExternalOutput"),
        "tl": nc.dram_tensor("tl", [H, 4], F32, kind="ExternalOutput"),
    }

    x2te_v = t_in["x2te"][:, :, :]
    x2n_v = t_in["x2n"][:, :, :]

    DR = mybir.MatmulPerfMode.DoubleRow

    with tile.TileContext(nc) as tc:
        with (
            tc.tile_pool(name="singles", bufs=1) as singles,
            tc.tile_pool(name="sa", bufs=1) as sa,
            tc.tile_pool(name="sb", bufs=1) as sbp,
            tc.tile_pool(name="psbig", bufs=4, space="PSUM") as psbig,
            tc.tile_pool(name="pstr", bufs=4, space="PSUM") as pstr,
        ):
            ep_targets = []

            xtr_a = sa.tile([J, 2, CE], E4, tag="xtr_a")
            ep_targets.append(nc.sync.dma_start(out=xtr_a, in_=x2te_v[:, 0:2, :]))
            xtr_b = sa.tile([J, 6, CE], E4, tag="xtr_b")
            ep_targets.append(nc.sync.dma_start(out=xtr_b, in_=x2te_v[:, 2:8, :]))
            xtr_c = sa.tile([J, 8, CE], E4, tag="xtr_c")
            ep_targets.append(nc.scalar.dma_start(out=xtr_c, in_=x2te_v[:, 8:16, :]))
            xn_a = sbp.tile([J, 2, C], E4, tag="xn_a")
            ep_targets.append(nc.sync.dma_start(out=xn_a, in_=x2n_v[:, 0:2, :]))
            xn_b = sbp.tile([J, 4, C], E4, tag="xn_b")
            ep_targets.append(nc.scalar.dma_start(out=xn_b, in_=x2n_v[:, 2:6, :]))
            xn_c = sbp.tile([J, 4, C], E4, tag="xn_c")
            ep_targets.append(nc.sync.dma_start(out=xn_c, in_=x2n_v[:, 6:10, :]))
            xn_d = sbp.tile([J, 4, C], E4, tag="xn_d")
            ep_targets.append(nc.scalar.dma_start(out=xn_d, in_=x2n_v[:, 10:14, :]))
            xn_e = sbp.tile([J, 2, C], E4, tag="xn_e")
            ep_targets.append(nc.sync.dma_start(out=xn_e, in_=x2n_v[:, 14:16, :]))

            ident16 = singles.tile([H, H], BF)
            nc.gpsimd.memset(ident16, 0.0)
            nc.gpsimd.affine_select(
                out=ident16,
                in_=ident16,
                compare_op=mybir.AluOpType.not_equal,
                fill=1.0,
                base=0,
                pattern=[[-1, H]],
                channel_multiplier=1,
            )
            ident32 = singles.tile([H2, H], F32)
            nc.gpsimd.memset(ident32, 0.0)
            nc.gpsimd.affine_select(
                out=ident32,
                in_=ident32,
                compare_op=mybir.AluOpType.not_equal,
                fill=1.0,
                base=0,
                pattern=[[-1, H]],
                channel_multiplier=1,
            )
            i_pool = nc.gpsimd.affine_select(
                out=ident32,
                in_=ident32,
                compare_op=mybir.AluOpType.not_equal,
                fill=1.0,
                base=-H,
                pattern=[[-1, H]],
                channel_multiplier=1,
            )
            ident32v = singles.tile([H2, H], F32R)
            nc.vector.tensor_copy(out=ident32v, in_=ident32)

            ps = [
                psbig.tile([H2, NB], F32, tag="big", name=f"ps_s{m}")
                for m in range(4)
            ]

            def xtr_tile(j):
                if j == 0:
                    return xtr_a, 0
                if j <= 3:
                    return xtr_b, 2 * (j - 1)
                return xtr_c, 2 * (j - 4)

            for j in range(8):
                xt, o = xtr_tile(j)
                for m in range(4):
                    nc.tensor.matmul(
                        ps[m][:H2, :],
                        lhsT=xt[:, o : o + 2, 0:RW],
                        rhs=xt[:, o : o + 2, RW + m * NB : RW + (m + 1) * NB],
                        start=(j == 0),
                        stop=(j == 7),
                        perf_mode=DR,
                    )

            ssb = singles.tile([H2, S_LOC], F32R)
            for m in range(4):
                nc.vector.tensor_copy(
                    out=ssb[:, m * NB : (m + 1) * NB], in_=ps[m][:H2, :]
                )
            mg = [
                psbig.tile([H2, NB], F32, tag="big", name=f"ps_mg{m}")
                for m in range(4)
            ]
            for m in range(4):
                nc.tensor.matmul(
                    mg[m][:H, :],
                    lhsT=ident32v[:, :],
                    rhs=ssb[:, m * NB : (m + 1) * NB],
                    start=True,
                    stop=True,
                )
            Psb = singles.tile([H, S_LOC], BF)
            tl_sb = singles.tile([H, 4], F32)
            i_exp = None
            for m in range(4):
                i_exp = nc.scalar.activation(
                    out=Psb[:, m * NB : (m + 1) * NB],
                    in_=mg[m][:H, :],
                    func=mybir.ActivationFunctionType.Exp,
                    scale=1.0 / SCALE_R,
                    bias=EXP_BIAS,
                    accum_out=tl_sb[:, m : m + 1],
                )

            PT = singles.tile([J, CH, H2], E4)
            i_dve = None
            for sb in range(CH):
                blk = slice(sb * J, (sb + 1) * J)
                pst = pstr.tile([J, H], BF, tag="tr", name=f"tr{sb}")
                nc.tensor.transpose(pst, Psb[:, blk], ident16)
                nc.vector.tensor_copy(out=PT[:, sb, 0:H], in_=pst)
                i_dve = nc.vector.tensor_sub(
                    out=PT[:, sb, H:H2], in0=pst, in1=PT[:, sb, 0:H]
                )

            pt = [
                psbig.tile([H2, NB], F32, tag="big", name=f"ps_t{m}")
                for m in range(4)
            ]

            def xn_tile(j):
                if j == 0:
                    return xn_a, 0
                if j <= 2:
                    return xn_b, 2 * (j - 1)
                if j <= 4:
                    return xn_c, 2 * (j - 3)
                if j <= 6:
                    return xn_d, 2 * (j - 5)
                return xn_e, 0

            i_pe = None
            for j in range(8):
                xn, o = xn_tile(j)
                for m in range(4):
                    i_pe = nc.tensor.matmul(
                        pt[m][:H2, :],
                        lhsT=PT[:, 2 * j : 2 * j + 2, :],
                        rhs=xn[:, o : o + 2, m * NB : (m + 1) * NB],
                        start=(j == 0),
                        stop=(j == 7),
                        perf_mode=DR,
                    )

            tt_sb = singles.tile([H2, C], F32)
            i_tcopies = []
            for m in range(2):
                i_tcopies.append(
                    nc.vector.tensor_copy(
                        out=tt_sb[:, m * NB : (m + 1) * NB], in_=pt[m][:H2, :]
                    )
                )
            for m in range(2, 4):
                i_tcopies.append(
                    nc.scalar.copy(
                        out=tt_sb[:, m * NB : (m + 1) * NB], in_=pt[m][:H2, :]
                    )
                )
            i_out_l = nc.gpsimd.dma_start(out=t_out["tl"][:, :], in_=tl_sb)
            i_out_t = nc.gpsimd.dma_start(
                out=t_out["tt"][:, 0 : 2 * NB], in_=tt_sb[:, 0 : 2 * NB]
            )
            i_out_t2 = nc.gpsimd.dma_start(
                out=t_out["tt"][:, 2 * NB :], in_=tt_sb[:, 2 * NB :]
            )
            i_gp_last = nc.gpsimd.nop(nofuse=True, hint="dep")

            ep_targets += [
                i_pool, i_exp, i_dve, i_pe, *i_tcopies,
                i_out_l, i_out_t, i_out_t2, i_gp_last,
            ]
            for t in ep_targets:
                n = nc.sync.nop(nofuse=True, hint="dep")
                add_dep_helper(n.ins, t.ins, reason="drain-funnel")

    return nc


_NC_CACHE = None


def _get_nc() -> bass.Bass:
    global _NC_CACHE
    if _NC_CACHE is None:
        _NC_CACHE = _build_program()
    return _NC_CACHE


def _prep_in_maps(x1, x2, Wq, Wk):
    x1 = np.asarray(x1, np.float32)
    x2 = np.asarray(x2, np.float32)
    Wq = np.asarray(Wq, np.float32)
    Wk = np.asarray(Wk, np.float32)

    q = (Wq @ x1) * INV_SQRT_K
    R = np.einsum("hj,hjc->ch", q.reshape(H, J), Wk.reshape(H, J, C))
    Rs = np.clip(R * SCALE_R, -240.0, 240.0).astype(np.float32)
    Rhi = Rs.astype(_FP8_NP)
    Rlo = (Rs - Rhi.astype(np.float32)).astype(_FP8_NP)
    Rhi_p = np.ascontiguousarray(Rhi.reshape(CH, J, H).transpose(1, 0, 2))
    Rlo_p = np.ascontiguousarray(Rlo.reshape(CH, J, H).transpose(1, 0, 2))

    in_maps = []
    for c in range(NCORES):
        shard = x2[c * S_LOC : (c + 1) * S_LOC]
        x2te = np.empty((J, CH, CE), dtype=_FP8_NP)
        x2te[:, :, 0:H] = Rhi_p
        x2te[:, :, H:RW] = Rlo_p
        x2te[:, :, RW:] = (
            shard.T.reshape(CH, J, S_LOC).transpose(1, 0, 2).astype(_FP8_NP)
        )
        x2n_c = np.ascontiguousarray(
            shard.reshape(CH, J, C).transpose(1, 0, 2)
        ).astype(_FP8_NP)
        in_maps.append({"x2te": x2te, "x2n": x2n_c})
    return in_maps


def _merge(results, Wv, Wo, bo):
    Wv = np.asarray(Wv, np.float32)
    Wo = np.asarray(Wo, np.float32)
    bo = np.asarray(bo, np.float32)
    t_tot = np.zeros((H, C), np.float64)
    l_tot = np.zeros(H, np.float64)
    for r in results:
        tt = r["tt"].astype(np.float64)
        t_tot += tt[:H] + tt[H:]
        l_tot += r["tl"].astype(np.float64).sum(axis=1)
    tn = t_tot / l_tot[:, None]
    u = np.einsum("hc,hjc->hj", tn, Wv.astype(np.float64).reshape(H, J, C))
    out = u.reshape(HJ) @ Wo.T.astype(np.float64) + bo.astype(np.float64)
    return out.astype(np.float32).reshape(1, ODIM)


def kernel(x1, x2, Wq, Wk, Wv, Wo, bo):
    nc = _get_nc()
    in_maps = _prep_in_maps(x1, x2, Wq, Wk)
    res = run_bass_kernel_spmd(nc, in_maps, list(range(NCORES)))
    return _merge(res.results, Wv, Wo, bo)


def run_traced(x1, x2, Wq, Wk, Wv, Wo, bo, **trace_kwargs):
    nc = _get_nc()
    in_maps = _prep_in_maps(x1, x2, Wq, Wk)
    res = run_bass_kernel_spmd(
        nc, in_maps, list(range(NCORES)), trace=True, **trace_kwargs
    )
    return _merge(res.results, Wv, Wo, bo), res
